# revision 1
# baseline (speedup 1.0000x reference)
"""Trainium2 Bass kernel for the CPC/moe_routing problem.

Strategy: the problem fully decomposes by category (the [N,N] negative-term
matrix is only needed where c_i == c_j).  We shard BY CATEGORY: 16 categories
across 8 cores = 2 categories/core.  Each core computes, for its rows only:
  f_x = relu(x@W1+b1)@W2+b2, f_z = z@Wz+bz, u = f_x @ w_s[cat]
  S = softplus(u @ f_z^T) per category block, neg_T = row-mean over the
  category, T = softplus(diag) via elementwise u*f_z,
  out = log(T+eps) - log(neg_T+eps)
On-chip layouts are transposed ([feature, row]) so matmuls contract along
partitions and biases are per-partition.  Matmul operands are fp16 (weights
host-rounded; activations device-rounded) with fp32 PSUM accumulation; the
second MLP layer is host-fused with the routing weights (W2c = W2 @ w_s[g]).

Numerical notes:
- negative-term sum uses softplus(v) ~= relu(v): with per-row |v| std >= 10
  on these inputs the dropped log1p(exp(-|v|)) term biases neg_T by <= 6e-3
  (~1e-4 relative), i.e. <~1e-3 absolute on the final log output.
- rows padded up to the per-category capacity P get z := z0 with
  z0 = -Wz^-T bz (host-solved), so their f_z is ~0 on device and they
  contribute ~nothing to the relu-sum; counts use the true 1/cnt from host.
- the positive term log(softplus(pos)+eps) is computed with an exact
  piecewise form (it is sensitive when pos is very negative).
"""

import math
from contextlib import ExitStack

import numpy as np

import concourse.bass as bass
import concourse.mybir as mybir
import concourse.tile as tile
from concourse import bacc
from concourse import bass_utils

F32 = mybir.dt.float32
F32R = mybir.dt.float32r
BF16 = mybir.dt.bfloat16
FP16 = mybir.dt.float16
AF = mybir.ActivationFunctionType
ALU = mybir.AluOpType

N, D_IN, HID, Z, C = 8192, 256, 512, 128, 16
N_CORES = 8
CATS_PER_CORE = C // N_CORES
EPS32 = float(np.float32(1e-16))
LNEPS = float(np.log(np.float64(np.float32(1e-16))))  # -36.8413614...
POS_THRESH = -9.0
N_WARMUP_MM = 28


def _col_tiles(total, step=512):
    tiles = []
    s = 0
    while s < total:
        nt = min(step, total - s)
        tiles.append((s, nt))
        s += nt
    return tiles


def build_program(P):
    """Build the single-core Bass/Tile program (SPMD: same NEFF on all cores)."""
    NCH = P // 128
    R = CATS_PER_CORE * P
    F = R // 128  # chunk-major columns of per-row [128, F] vectors
    TIL = _col_tiles(P)
    RTIL = _col_tiles(R)

    nc = bacc.Bacc(
        "TRN2",
        target_bir_lowering=False,
        debug=False,
        enable_asserts=False,
        num_devices=N_CORES,
    )

    xT = nc.dram_tensor("xT", [2, 128, R], FP16, kind="ExternalInput")
    zT = nc.dram_tensor("zT", [128, R], FP16, kind="ExternalInput")
    W1 = nc.dram_tensor("W1", [2, 128, HID], FP16, kind="ExternalInput")
    W2c = nc.dram_tensor("W2c", [CATS_PER_CORE, 4, 128, Z], FP16, kind="ExternalInput")
    Wz = nc.dram_tensor("Wz", [Z, Z], FP16, kind="ExternalInput")
    b1 = nc.dram_tensor("b1", [128, 4], F32, kind="ExternalInput")
    b2c = nc.dram_tensor("b2c", [128, CATS_PER_CORE], F32, kind="ExternalInput")
    bz = nc.dram_tensor("bz", [128, 1], F32, kind="ExternalInput")
    cstd = nc.dram_tensor("cst", [128, 1], F32R, kind="ExternalInput")
    invd = nc.dram_tensor("invd", [128, F], F32, kind="ExternalInput")
    outd = nc.dram_tensor("out", [128, F], F32, kind="ExternalOutput")

    with tile.TileContext(nc) as tc, ExitStack() as ctx:
        perm = ctx.enter_context(tc.tile_pool(name="perm", bufs=1))
        vec = ctx.enter_context(tc.tile_pool(name="vec", bufs=1))

        # ---- PE warm-up: keep the HAM activity monitor busy while DMAs run,
        # so real matmuls start (and stay) at 2.4 GHz instead of 1.2 GHz.
        with (
            tc.tile_pool(name="warm", bufs=1) as warm,
            tc.tile_pool(name="pswarm", bufs=1, space="PSUM") as pswarm,
        ):
            wdum = warm.tile([128, 256], BF16)
            nc.gpsimd.memset(wdum[:], 0.5)
            pdum = pswarm.tile([16, 256], F32)
            for _ in range(N_WARMUP_MM):
                nc.tensor.matmul(
                    pdum[:], wdum[:, 0:16], wdum[:], start=True, stop=True
                )

        # ---- persistent weights / constants ----
        # W1/b1 first: the first row-tile's matmuls only need these, so the
        # PE can start while the rest of the weights stream in.
        sbW1 = perm.tile([128, 2, HID], FP16)
        for f in range(2):
            nc.scalar.dma_start(sbW1[:, f, :], W1[f])
        sbb1 = perm.tile([128, 4], F32)
        nc.scalar.dma_start(sbb1[:], b1[:])
        sbW2c = perm.tile([128, CATS_PER_CORE, 4, Z], FP16)
        for g in range(CATS_PER_CORE):
            for q in range(4):
                nc.scalar.dma_start(sbW2c[:, g, q, :], W2c[g, q])
        sbb2c = perm.tile([128, CATS_PER_CORE], F32)
        nc.scalar.dma_start(sbb2c[:], b2c[:])
        sbWz = perm.tile([128, Z], FP16)
        sbbz = perm.tile([128, 1], F32)
        sbones = perm.tile([128, 1], F32R)
        sbinv = perm.tile([128, F], F32)
        sbeps = perm.tile([128, 1], F32)
        nc.gpsimd.memset(sbeps[:], EPS32)

        def load_rest_of_weights():
            nc.scalar.dma_start(sbWz[:], Wz[:])
            nc.scalar.dma_start(sbbz[:], bz[:])
            nc.scalar.dma_start(sbones[:], cstd[:])
            nc.scalar.dma_start(sbinv[:], invd[:])

        # ---- persistent activations ----
        sbfz = perm.tile([128, R], F32R)
        sbfzh = perm.tile([128, R], FP16)
        sbu = perm.tile([128, R], FP16)
        sbprod = perm.tile([128, R], F32R)
        nacc = perm.tile([128, F], F32)  # per-row relu-sum accumulators

        load_rest_of_weights()

        # ======== Stage B: MLP + f_z over row tiles; u per category ========
        with (
            tc.tile_pool(name="xin", bufs=4) as xin,
            tc.tile_pool(name="hrelu", bufs=2) as hpool,
            tc.tile_pool(name="psB", bufs=1, space="PSUM") as psB,
            tc.tile_pool(name="psB1", bufs=1, space="PSUM") as psB1,
            tc.tile_pool(name="psp", bufs=1, space="PSUM") as psp,
        ):
            pspos = psp.tile([128, F], F32)
            for (ts, nt) in RTIL:
                sl = slice(ts, ts + nt)
                xt = xin.tile([128, 2, nt], FP16, tag="xt")
                for f in range(2):
                    nc.sync.dma_start(xt[:, f, :], xT[f, :, sl])
                zt = xin.tile([128, nt], FP16, tag="zt")
                nc.sync.dma_start(zt[:], zT[:, sl])

                ph = psB.tile([128, 4, nt], F32, tag="ph")
                for h in range(4):
                    hs = slice(h * 128, (h + 1) * 128)
                    for f in range(2):
                        nc.tensor.matmul(
                            ph[:, h, :],
                            sbW1[:, f, hs],
                            xt[:, f, :],
                            start=(f == 0),
                            stop=(f == 1),
                        )
                ht = hpool.tile([128, 4, nt], FP16, tag="ht")
                for h in range(4):
                    # ht = relu(ph + b1)  (ACT: per-partition bias is free)
                    nc.scalar.activation(
                        ht[:, h, :], ph[:, h, :], AF.Relu, bias=sbb1[:, h : h + 1]
                    )

                pfz = psB1.tile([128, nt], F32, tag="pfz", bufs=2)
                nc.tensor.matmul(pfz[:], sbWz[:], zt[:], start=True, stop=True)
                nc.vector.tensor_scalar_add(sbfz[:, sl], pfz[:], sbbz[:, 0:1])
                nc.vector.tensor_scalar_add(sbfzh[:, sl], pfz[:], sbbz[:, 0:1])

                # u directly from h via the host-fused W2c = W2 @ w_s[cat]
                # (split the row range at category boundaries).  The
                # positive-term pos[p, c] = prod[:, c*128+p] . ones lands
                # directly in chunk-major [128, F] layout by using the prod
                # block as the STATIONARY operand.
                s0 = ts
                while s0 < ts + nt:
                    g = s0 // P
                    e0 = min(ts + nt, (g + 1) * P)
                    cn = e0 - s0
                    slc = slice(s0, e0)
                    pu = psB1.tile([128, cn], F32, tag="pu", name=f"pu_{s0}")
                    for q in range(4):
                        nc.tensor.matmul(
                            pu[:],
                            sbW2c[:, g, q, :],
                            ht[:, q, s0 - ts : e0 - ts],
                            start=(q == 0),
                            stop=(q == 3),
                        )
                    b2g = sbb2c[:, g : g + 1]
                    nc.vector.tensor_scalar_add(sbu[:, slc], pu[:], b2g)
                    nc.vector.scalar_tensor_tensor(
                        sbprod[:, slc], pu[:], b2g, sbfz[:, slc],
                        op0=ALU.add, op1=ALU.mult,
                    )
                    for cc in range(cn // 128):
                        col = s0 // 128 + cc
                        c0 = s0 + cc * 128
                        # N=1 violates fp32r ISA rules; plain fp32 is fine
                        # here (cost is the ~60-cycle floor anyway)
                        nc.tensor.matmul(
                            pspos[:, col : col + 1],
                            sbprod[:, c0 : c0 + 128].bitcast(F32),
                            sbones[:].bitcast(F32),
                            start=True, stop=True,
                        )
                    s0 = e0

            tpos = vec.tile([128, F], F32)
            nc.vector.tensor_copy(tpos[:], pspos[:])

        # ======== positive-term log-space chain (overlaps the neg loop) =====

        # ACT set 1 (exp_and_others: Abs/Exp), then set 2 (natural_log: Ln)
        t_ax = vec.tile([128, F], F32)
        i_ax = nc.scalar.activation(t_ax[:], tpos[:], AF.Abs)
        t_y = vec.tile([128, F], F32)
        nc.vector.tensor_scalar_add(t_y[:], tpos[:], -LNEPS)
        t_ay = vec.tile([128, F], F32)
        i_ay = nc.scalar.activation(t_ay[:], t_y[:], AF.Abs)
        t_e2 = vec.tile([128, F], F32)
        i_e2 = nc.scalar.activation(t_e2[:], t_ax[:], AF.Exp, scale=-1.0)
        t_e1 = vec.tile([128, F], F32)
        i_e1 = nc.scalar.activation(t_e1[:], t_ay[:], AF.Exp, scale=-1.0)
        t_r2 = vec.tile([128, F], F32)
        nc.vector.tensor_scalar_max(t_r2[:], tpos[:], 0.0)
        t_r1 = vec.tile([128, F], F32)
        nc.vector.tensor_scalar_max(t_r1[:], t_y[:], 0.0)
        t_l2 = vec.tile([128, F], F32)
        i_l2 = nc.scalar.activation(t_l2[:], t_e2[:], AF.Ln, bias=1.0)
        t_l1 = vec.tile([128, F], F32)
        i_l1 = nc.scalar.activation(t_l1[:], t_e1[:], AF.Ln, bias=1.0)
        # batch ACT ops by table set: Abs/Exp (resident set), then the Lns
        tile.add_dep_helper(i_e2.ins, i_ay.ins, sync=False, reason="act batch")
        tile.add_dep_helper(i_l2.ins, i_e1.ins, sync=False, reason="act batch")
        t_sp = vec.tile([128, F], F32)
        nc.vector.tensor_add(t_sp[:], t_r2[:], t_l2[:])
        t_p2 = vec.tile([128, F], F32)
        i_p2 = nc.scalar.activation(t_p2[:], t_sp[:], AF.Ln, bias=sbeps[:])
        tile.add_dep_helper(i_p2.ins, i_l1.ins, sync=False, reason="act batch")
        t_p1 = vec.tile([128, F], F32)
        nc.vector.scalar_tensor_tensor(
            t_p1[:], t_r1[:], LNEPS, t_l1[:], op0=ALU.add, op1=ALU.add
        )
        t_m = vec.tile([128, F], mybir.dt.int32)
        nc.vector.tensor_scalar(t_m[:], tpos[:], POS_THRESH, None, op0=ALU.is_lt)
        t_posln = vec.tile([128, F], F32)
        nc.vector.select(t_posln[:], t_m[:], t_p1[:], t_p2[:])

        # ======== Stage C: negative sums ========
        with (
            tc.tile_pool(name="junkp", bufs=2) as jpool,
            tc.tile_pool(name="psm", bufs=2, space="PSUM") as psm,
        ):
            # per category, per 128-row i-chunk:
            #   M'[i, j] = u_i . f_z_j for all j; nacc[:, chunk] = sum_j relu
            for g in range(CATS_PER_CORE):
                for ic in range(NCH):
                    ucol = g * P + ic * 128
                    pm = psm.tile([128, P], F32, tag="pm")
                    for (ts, nt) in TIL:
                        nc.tensor.matmul(
                            pm[:, ts : ts + nt],
                            sbu[:, ucol : ucol + 128],
                            sbfzh[:, g * P + ts : g * P + ts + nt],
                            start=True, stop=True,
                        )
                    junk = jpool.tile([128, P], F32, tag="junk")
                    col = g * NCH + ic
                    nc.vector.tensor_scalar(
                        junk[:], pm[:], 0.0, 0.0, op0=ALU.max, op1=ALU.add,
                        accum_out=nacc[:, col : col + 1],
                    )


        # ======== final combination ========
        t_negT = vec.tile([128, F], F32)
        nc.vector.tensor_mul(t_negT[:], nacc[:], sbinv[:])
        t_lnneg = vec.tile([128, F], F32)
        i_lnneg = nc.scalar.activation(t_lnneg[:], t_negT[:], AF.Ln, bias=sbeps[:])
        # keep the Ln-set ops together: lnneg must not jump ahead of the
        # pos-chain Lns or the ACT table set gets reloaded twice
        tile.add_dep_helper(
            i_lnneg.ins, i_p2.ins, sync=False, reason="act table order"
        )

        t_out = vec.tile([128, F], F32)
        nc.vector.tensor_sub(t_out[:], t_posln[:], t_lnneg[:])
        nc.sync.dma_start(outd[:], t_out[:])

    nc.compile()
    return nc


def prepare(x, c, z, W1, b1, W2, b2, Wz, bz, w_s):
    """Host-side sharding: returns (P, in_maps, slots, idx)."""
    x = np.ascontiguousarray(np.asarray(x, dtype=np.float32))
    z = np.ascontiguousarray(np.asarray(z, dtype=np.float32))
    W1 = np.asarray(W1, dtype=np.float32)
    b1 = np.asarray(b1, dtype=np.float32)
    W2 = np.asarray(W2, dtype=np.float32)
    b2 = np.asarray(b2, dtype=np.float32)
    Wz = np.asarray(Wz, dtype=np.float32)
    bz = np.asarray(bz, dtype=np.float32)
    w_s = np.asarray(w_s, dtype=np.float32)
    ci = np.asarray(c).astype(np.int64)

    idx = [np.nonzero(ci == g)[0] for g in range(C)]
    cnt = np.array([len(i) for i in idx])
    P = 128 * max(1, math.ceil(cnt.max() / 128))
    NCH = P // 128
    R = CATS_PER_CORE * P
    F = R // 128

    # padded rows get z0 with Wz^T z0 + bz = 0, so their f_z vanishes on
    # device (solve against the fp16-rounded Wz the device actually uses)
    z0 = -np.linalg.solve(
        Wz.astype(np.float16).astype(np.float64).T, bz.astype(np.float64)
    )
    z0 = z0.astype(np.float32)

    W1h = np.ascontiguousarray(W1.reshape(2, 128, HID).astype(np.float16))
    b1h = np.ascontiguousarray(b1.reshape(4, 128).T)  # [128, 4]
    bzh = np.ascontiguousarray(bz.reshape(128, 1))
    cst_arr = np.ones((128, 1), dtype=np.float32)
    Wzh = np.ascontiguousarray(Wz.astype(np.float16))
    # host-fused second layer: W2c[g] = W2 @ w_s[g], b2c[g] = b2 @ w_s[g]
    W2c_all = np.einsum(
        "hd,cde->che", W2.astype(np.float64), w_s.astype(np.float64)
    )  # [C, HID, Z]
    b2c_all = np.einsum(
        "d,cde->ce", b2.astype(np.float64), w_s.astype(np.float64)
    )  # [C, Z]

    in_maps = []
    slots = []
    for k in range(N_CORES):
        cats = [CATS_PER_CORE * k + j for j in range(CATS_PER_CORE)]
        padded = []
        inv_chunk = np.zeros((128, F), dtype=np.float32)
        pad_flags = np.zeros(R, dtype=bool)
        for j, g in enumerate(cats):
            n_real = cnt[g]
            pad_to = P - n_real
            fill = idx[g][0] if n_real > 0 else 0
            padded.append(
                np.concatenate([idx[g], np.full(pad_to, fill, dtype=idx[g].dtype)])
            )
            pad_flags[j * P + n_real : (j + 1) * P] = True
            inv_chunk[:, j * NCH : (j + 1) * NCH] = 1.0 / max(n_real, 1)
        rows = np.concatenate(padded)  # [R] global row indices
        xTk = np.ascontiguousarray(x[rows].T.reshape(2, 128, R).astype(np.float16))
        zk = z[rows].copy()
        zk[pad_flags] = z0[None, :, 0] if z0.ndim == 2 else z0
        zTk = np.ascontiguousarray(zk.T.astype(np.float16))
        W2ck = np.ascontiguousarray(
            W2c_all[cats].reshape(CATS_PER_CORE, 4, 128, Z).astype(np.float16)
        )
        b2ck = np.ascontiguousarray(
            b2c_all[cats].T.astype(np.float32)
        )  # [128, CATS_PER_CORE]
        in_maps.append(
            {
                "xT": xTk,
                "zT": zTk,
                "W1": W1h,
                "W2c": W2ck,
                "Wz": Wzh,
                "b1": b1h,
                "b2c": b2ck,
                "bz": bzh,
                "cst": cst_arr,
                "invd": inv_chunk,
            }
        )
        slots.append((cats, [cnt[g] for g in cats]))
    return P, in_maps, slots, idx


def gather_output(P, slots, idx, core_outs):
    NCH = P // 128
    out_full = np.zeros(N, dtype=np.float32)
    for k in range(N_CORES):
        om = core_outs[k]  # [128, F], out[p, g*NCH+r] = row g*P + r*128 + p
        cats, counts = slots[k]
        for j, g in enumerate(cats):
            rows_cat = om[:, j * NCH : (j + 1) * NCH].T.reshape(P)
            n_real = counts[j]
            if n_real:
                out_full[idx[g]] = rows_cat[:n_real]
    return out_full


def kernel(x, c, z, W1, b1, W2, b2, Wz, bz, w_s):
    P, in_maps, slots, idx = prepare(x, c, z, W1, b1, W2, b2, Wz, bz, w_s)
    nc = build_program(P)
    res = bass_utils.run_bass_kernel_spmd(nc, in_maps, core_ids=list(range(N_CORES)))
    return gather_output(P, slots, idx, [r["out"] for r in res.results])



# revision 6
# speedup vs baseline: 1.3031x; 1.3031x over previous
"""Trainium2 Bass kernel for the CPC/moe_routing problem.

Category-sharded SPMD: 16 categories across 8 cores, 2 per core (paired
big+small by count so the compiled per-slot capacities P0 >= P1 are tight).
Each core, for its rows only:
  f_x = relu(x@W1+b1)@W2+b2 (second layer host-fused with w_s[cat]),
  f_z = Wz^T z'   (z' host-shifted so the bias is exact and pad rows give 0),
  u = f_x @ w_s[cat],  M = u @ f_z^T per category,
  neg_T = row-mean relu(M) (softplus~=relu, |M| large), T = softplus(u.f_z),
  out = log(T+eps) - log(neg_T+eps)  (exact piecewise log-softplus).

Perf structure (vs the 50us baseline):
- 7 big DMA instructions instead of 26 (issue cost ~600ns each on the two
  HWDGE engines), issued at the top of the body on both queues.
- PE warm-up reads a persistent-pool buffer so no SBUF-reuse dependency
  blocks the x/z DMA issue; warm-up length covers the DMA arrival window
  and the 3us HAM clock ramp, then the real matmul stream starts with no
  gap (PE stays at 2.4 GHz).
- pos term = colsum of q = u*fzh (fp16, pool engine) via tiny N=1 matmuls
  interleaved into the neg phase; no f32r prod pass, no second f_z copy.
- relu split ACT/DVE per h-chunk; neg relu-row-sums alternate DVE/ACT with
  accum_out; gpsimd takes the SBUF-side glue (q, pos-chain scalar ops).
- PSUM fits 8 banks exactly: warm(1)->reused, pfz(3), ph(2x2), pu(1), then
  the C phase reuses all: pm(2x2), junk(2), pspos(1).
"""

import math
from contextlib import ExitStack

import numpy as np

import concourse.bass as bass
import concourse.mybir as mybir
import concourse.tile as tile
from concourse import bacc
from concourse import bass_utils
from concourse import hw_specs as _hw_specs

# All activation funcs used here (Relu/Copy/Abs/Exp/Ln) live in the single
# "natural_log_exp_and_others" table set, but the greedy table-load pass
# would pick exp_and_others first and then swap to natural_log mid-kernel
# (1.28us on the ACT engine, on the critical path of the output tail).
# Restrict the pass's choices to the one covering set; ids stay original.
_MONO_TABLE = "natural_log_exp_and_others"


def _mono_tables(arch):
    tabs = _hw_specs.get_activation_tables(arch)
    if _MONO_TABLE not in tabs:
        return tabs
    return {k: (v if k == _MONO_TABLE else set()) for k, v in tabs.items()}


bacc.get_activation_tables = _mono_tables

F32 = mybir.dt.float32
BF16 = mybir.dt.bfloat16
FP16 = mybir.dt.float16
AF = mybir.ActivationFunctionType
ALU = mybir.AluOpType

N, D_IN, HID, Z, C = 8192, 256, 512, 128, 16
N_CORES = 8
EPS32 = float(np.float32(1e-16))
LNEPS = float(np.log(np.float64(np.float32(1e-16))))  # -36.8413614...
POS_THRESH = -9.0
N_WARMUP_MM = 30


def _tiles(start, total, step):
    out = []
    s = 0
    while s < total:
        nt = min(step, total - s)
        out.append((start + s, nt))
        s += nt
    return out


def build_program(P0, P1):
    NCH0, NCH1 = P0 // 128, P1 // 128
    R = P0 + P1
    F = NCH0 + NCH1
    PS = (P0, P1)
    SOFF = (0, P0)
    NCHS = (NCH0, NCH1)
    K = 7 + F  # consts cols: b1[4], b2c[2], eps[1], invd[F]
    ICOL = 7

    nc = bacc.Bacc(
        "TRN2",
        target_bir_lowering=False,
        debug=False,
        enable_asserts=False,
        num_devices=N_CORES,
    )

    xT = nc.dram_tensor("xT", [128, 2, R], FP16, kind="ExternalInput")
    zT = nc.dram_tensor("zT", [128, R], FP16, kind="ExternalInput")
    wz1 = nc.dram_tensor("wz1", [128, 128 + 2 * HID], FP16, kind="ExternalInput")
    w2c = nc.dram_tensor("w2c", [128, 2, 4, Z], FP16, kind="ExternalInput")
    cst = nc.dram_tensor("cst", [128, K], F32, kind="ExternalInput")
    outd = nc.dram_tensor("out", [128, F], F32, kind="ExternalOutput")

    with tile.TileContext(nc) as tc, ExitStack() as ctx:
        perm = ctx.enter_context(tc.tile_pool(name="perm", bufs=1))

        # ---- persistent SBUF ----
        sbxt = perm.tile([128, 2, R], FP16)
        sbzt = perm.tile([128, R], FP16)
        sbwz1 = perm.tile([128, 128 + 2 * HID], FP16)
        sbw2c = perm.tile([128, 2, 4, Z], FP16)
        sbcst = perm.tile([128, K], F32)
        sbfzh = perm.tile([128, R], FP16)
        sbu = perm.tile([128, R], FP16)
        sbq = perm.tile([128, R], FP16)
        sbht = perm.tile([128, 2, 4, 256], FP16)  # double-buffered relu out
        nacc = perm.tile([128, F], F32)
        junkD = perm.tile([128, P0], FP16)
        sbones = perm.tile([128, 1], FP16)
        wdum = perm.tile([128, 128], BF16)

        # ---- DMAs first: both HWDGE queues start filling immediately ----
        nc.sync.dma_start(sbzt[:], zT[:])
        nc.sync.dma_start(sbxt[:, :, 0:256], xT[:, :, 0:256])
        nc.sync.dma_start(sbxt[:, :, 256:P0], xT[:, :, 256:P0])
        nc.sync.dma_start(sbxt[:, :, P0:R], xT[:, :, P0:R])
        nc.scalar.dma_start(sbwz1[:], wz1[:])
        nc.scalar.dma_start(sbw2c[:], w2c[:])
        nc.scalar.dma_start(sbcst[:], cst[:])

        nc.gpsimd.memset(wdum[:], 0.5)
        nc.gpsimd.memset(sbones[:], 1.0)

        sbWz = sbwz1[:, 0:128]

        def sbW1(f, h):
            s = 128 + f * HID + h * 128
            return sbwz1[:, s : s + 128]

        # ---- PE warm-up (HAM clock ramp; covers the DMA arrival window) ----
        with tc.tile_pool(name="pswarm", bufs=1, space="PSUM") as pswarm:
            pdum = pswarm.tile([16, 128], F32)
            for _ in range(N_WARMUP_MM):
                nc.tensor.matmul(pdum[:], wdum[:, 0:16], wdum[:], start=True, stop=True)

        with (
            tc.tile_pool(name="pfzp", bufs=1, space="PSUM") as pfzp,
            tc.tile_pool(name="php", bufs=2, space="PSUM") as php,
            tc.tile_pool(name="pup", bufs=1, space="PSUM") as pup,
        ):
            # ---- f_z for both slots (z' is bias-baked; pads give exact 0) ----
            pfzA = pfzp.tile([128, P0], F32)
            pfzB = pfzp.tile([128, P1], F32)
            for (ts, nt) in _tiles(0, P0, 512):
                nc.tensor.matmul(
                    pfzA[:, ts : ts + nt], sbWz, sbzt[:, ts : ts + nt],
                    start=True, stop=True,
                )
            for (ts, nt) in _tiles(0, P1, 512):
                nc.tensor.matmul(
                    pfzB[:, ts : ts + nt], sbWz, sbzt[:, P0 + ts : P0 + ts + nt],
                    start=True, stop=True,
                )
            nc.scalar.activation(sbfzh[:, 0:P0], pfzA[:], AF.Copy)
            nc.scalar.activation(sbfzh[:, P0:R], pfzB[:], AF.Copy)

            # ---- MLP over column tiles; one-tile lookahead on PE ----
            tiles = []
            for s in range(2):
                tiles += [(s, ts, nt) for (ts, nt) in _tiles(SOFF[s], PS[s], 256)]
            NT = len(tiles)

            ph_of = {}
            ht_of = {}

            def emit_l1(i):
                s, ts, nt = tiles[i]
                ph = php.tile([128, 4, nt], F32, tag="ph", name=f"ph_{i}")
                ph_of[i] = ph
                for h in range(4):
                    for f in range(2):
                        nc.tensor.matmul(
                            ph[:, h, :],
                            sbW1(f, h),
                            sbxt[:, f, ts : ts + nt],
                            start=(f == 0),
                            stop=(f == 1),
                        )

            def emit_relu(i):
                s, ts, nt = tiles[i]
                ph = ph_of[i]
                ht = sbht[:, i % 2, :, 0:nt]
                ht_of[i] = ht
                for h in range(4):
                    b1h = sbcst[:, h : h + 1]
                    if h < 2:
                        nc.scalar.activation(ht[:, h, :], ph[:, h, :], AF.Relu, bias=b1h)
                    else:
                        nc.vector.tensor_scalar(
                            ht[:, h, :], ph[:, h, :], b1h, 0.0,
                            op0=ALU.add, op1=ALU.max,
                        )

            def emit_l2(i):
                s, ts, nt = tiles[i]
                ht = ht_of[i]
                pu = pup.tile([128, nt], F32, tag="pu", name=f"pu_{i}")
                for q in range(4):
                    nc.tensor.matmul(
                        pu[:],
                        sbw2c[:, s, q, :],
                        ht[:, q, :],
                        start=(q == 0),
                        stop=(q == 3),
                    )
                nc.vector.tensor_scalar_add(
                    sbu[:, ts : ts + nt], pu[:], sbcst[:, 4 + s : 5 + s]
                )

            emit_l1(0)
            emit_relu(0)
            for i in range(1, NT):
                emit_l1(i)
                emit_relu(i)
                emit_l2(i - 1)
            emit_l2(NT - 1)

            # q = u * fzh per slot on the pool engine (SBUF-only)
            nc.gpsimd.tensor_tensor(
                sbq[:, 0:P0], sbu[:, 0:P0], sbfzh[:, 0:P0], op=ALU.mult
            )
            nc.gpsimd.tensor_tensor(
                sbq[:, P0:R], sbu[:, P0:R], sbfzh[:, P0:R], op=ALU.mult
            )

        # ======== Stage C: neg sums + pos columns ========
        with (
            tc.tile_pool(name="psm", bufs=2, space="PSUM") as psm,
            tc.tile_pool(name="junkp", bufs=1, space="PSUM") as junkp,
            tc.tile_pool(name="pspp", bufs=1, space="PSUM") as pspp,
        ):
            junkP = junkp.tile([128, P0], F32)
            pspos = pspp.tile([128, 16], F32)

            blocks = [(0, ic) for ic in range(NCH0)] + [(1, ic) for ic in range(NCH1)]

            def emit_pos(col):
                c0 = col * 128
                nc.tensor.matmul(
                    pspos[:, col : col + 1],
                    sbq[:, c0 : c0 + 128],
                    sbones[:],
                    start=True, stop=True,
                )

            for b, (s, ic) in enumerate(blocks):
                ucol = SOFF[s] + ic * 128
                pmt = psm.tile([128, P0], F32, tag="pm", name=f"pm_{b}")
                pm = pmt[:, 0 : PS[s]]
                for (ts, nt) in _tiles(SOFF[s], PS[s], 512):
                    nc.tensor.matmul(
                        pm[:, ts - SOFF[s] : ts - SOFF[s] + nt],
                        sbu[:, ucol : ucol + 128],
                        sbfzh[:, ts : ts + nt],
                        start=True, stop=True,
                    )
                # slot-0 pos columns ride along during slot-1 neg blocks;
                # slot-1 pos columns at the end.
                if s == 1:
                    emit_pos(ic)
                col = NCH0 + ic if s == 1 else ic
                if b % 2 == 0:
                    nc.vector.tensor_scalar(
                        junkD[:, 0 : PS[s]], pm[:], 0.0, 0.0,
                        op0=ALU.max, op1=ALU.add,
                        accum_out=nacc[:, col : col + 1],
                    )
                else:
                    nc.scalar.activation(
                        junkP[:, 0 : PS[s]], pm[:], AF.Relu,
                        accum_out=nacc[:, col : col + 1],
                    )
            for ic in range(NCH1, NCH0):
                emit_pos(ic)
            for ic in range(NCH1):
                emit_pos(NCH0 + ic)

            # ======== tail: neg log + exact piecewise log-softplus(pos) ======
            vec = ctx.enter_context(tc.tile_pool(name="vec", bufs=1))
            sbeps = sbcst[:, 6:7]

            t_neg = vec.tile([128, F], F32)
            nc.vector.tensor_mul(t_neg[:], nacc[:], sbcst[:, ICOL : ICOL + F])
            t_lnneg = vec.tile([128, F], F32)
            i_lnneg = nc.scalar.activation(t_lnneg[:], t_neg[:], AF.Ln, bias=sbeps)

            tpos = vec.tile([128, F], F32)
            nc.vector.tensor_copy(tpos[:], pspos[:, 0:F])

            t_ax = vec.tile([128, F], F32)
            i_ax = nc.scalar.activation(t_ax[:], tpos[:], AF.Abs)
            t_y = vec.tile([128, F], F32)
            nc.vector.tensor_scalar_add(t_y[:], tpos[:], -LNEPS)
            t_ay = vec.tile([128, F], F32)
            i_ay = nc.scalar.activation(t_ay[:], t_y[:], AF.Abs)
            t_e2 = vec.tile([128, F], F32)
            i_e2 = nc.scalar.activation(t_e2[:], t_ax[:], AF.Exp, scale=-1.0)
            t_e1 = vec.tile([128, F], F32)
            i_e1 = nc.scalar.activation(t_e1[:], t_ay[:], AF.Exp, scale=-1.0)
            t_r2 = vec.tile([128, F], F32)
            nc.vector.tensor_scalar_max(t_r2[:], tpos[:], 0.0)
            t_r1 = vec.tile([128, F], F32)
            nc.vector.tensor_scalar_max(t_r1[:], t_y[:], 0.0)
            t_l2 = vec.tile([128, F], F32)
            i_l2 = nc.scalar.activation(t_l2[:], t_e2[:], AF.Ln, bias=1.0)
            t_l1 = vec.tile([128, F], F32)
            i_l1 = nc.scalar.activation(t_l1[:], t_e1[:], AF.Ln, bias=1.0)
            t_sp = vec.tile([128, F], F32)
            nc.vector.tensor_add(t_sp[:], t_r2[:], t_l2[:])
            t_p2 = vec.tile([128, F], F32)
            i_p2 = nc.scalar.activation(t_p2[:], t_sp[:], AF.Ln, bias=sbeps)
            t_p1 = vec.tile([128, F], F32)
            nc.vector.scalar_tensor_tensor(
                t_p1[:], t_r1[:], LNEPS, t_l1[:], op0=ALU.add, op1=ALU.add
            )
            t_m = vec.tile([128, F], mybir.dt.int32)
            nc.vector.tensor_scalar(t_m[:], tpos[:], POS_THRESH, None, op0=ALU.is_lt)
            t_posln = vec.tile([128, F], F32)
            nc.vector.select(t_posln[:], t_m[:], t_p1[:], t_p2[:])

            t_out = vec.tile([128, F], F32)
            nc.vector.tensor_sub(t_out[:], t_posln[:], t_lnneg[:])
            nc.sync.dma_start(outd[:], t_out[:])

    nc.compile()
    return nc


def prepare(x, c, z, W1, b1, W2, b2, Wz, bz, w_s):
    x = np.ascontiguousarray(np.asarray(x, dtype=np.float32))
    z = np.ascontiguousarray(np.asarray(z, dtype=np.float32))
    W1 = np.asarray(W1, dtype=np.float32)
    b1 = np.asarray(b1, dtype=np.float32)
    W2 = np.asarray(W2, dtype=np.float32)
    b2 = np.asarray(b2, dtype=np.float32)
    Wz = np.asarray(Wz, dtype=np.float32)
    bz = np.asarray(bz, dtype=np.float32)
    w_s = np.asarray(w_s, dtype=np.float32)
    ci = np.asarray(c).astype(np.int64)

    idx = [np.nonzero(ci == g)[0] for g in range(C)]
    cnt = np.array([len(i) for i in idx])
    order = np.argsort(-cnt, kind="stable")
    # core k gets (order[k], order[15-k]); slot capacities from the global
    # extremes so the same NEFF fits every core tightly.
    P0 = 128 * max(1, math.ceil(cnt[order[0]] / 128))
    P1 = 128 * max(1, math.ceil(cnt[order[N_CORES]] / 128))
    NCH0, NCH1 = P0 // 128, P1 // 128
    R = P0 + P1
    F = NCH0 + NCH1
    K = 7 + F

    # z' = z - z0 so that Wz16^T z' = Wz^T z + bz exactly on device; pad
    # rows use z'=0 giving f_z = 0 exactly.
    Wz16 = Wz.astype(np.float16).astype(np.float64)
    z0 = np.linalg.solve(Wz16.T, -bz.astype(np.float64)).astype(np.float32)

    W1h = np.ascontiguousarray(
        W1.reshape(2, 128, HID).transpose(1, 0, 2).reshape(128, 2 * HID)
    ).astype(np.float16)
    wz1h = np.concatenate([Wz.astype(np.float16), W1h], axis=1)  # [128, 128+1024]

    W2c_all = np.einsum(
        "hd,cde->che", W2.astype(np.float64), w_s.astype(np.float64)
    )  # [C, HID, Z]
    b2c_all = np.einsum("d,cde->ce", b2.astype(np.float64), w_s.astype(np.float64))

    in_maps = []
    slots = []
    for k in range(N_CORES):
        cats = (int(order[k]), int(order[2 * N_CORES - 1 - k]))
        caps = (P0, P1)
        rows = []
        padf = []
        inv = np.zeros((128, F), dtype=np.float32)
        colbase = 0
        for s, g in enumerate(cats):
            n = cnt[g]
            fill = idx[g][0] if n > 0 else 0
            rows.append(
                np.concatenate([idx[g], np.full(caps[s] - n, fill, dtype=np.int64)])
            )
            pf = np.zeros(caps[s], dtype=bool)
            pf[n:] = True
            padf.append(pf)
            nch = caps[s] // 128
            inv[:, colbase : colbase + nch] = 1.0 / max(n, 1)
            colbase += nch
        rows = np.concatenate(rows)
        padf = np.concatenate(padf)

        xk = x[rows]  # [R, 256]
        xTk = np.ascontiguousarray(
            xk.T.reshape(2, 128, R).transpose(1, 0, 2)
        ).astype(np.float16)  # [128, 2, R]
        zk = z[rows] - z0[None, :]
        zk[padf] = 0.0
        zTk = np.ascontiguousarray(zk.T).astype(np.float16)  # [128, R]

        w2ck = np.zeros((128, 2, 4, Z), dtype=np.float16)
        for s, g in enumerate(cats):
            w2ck[:, s] = (
                W2c_all[g].reshape(4, 128, Z).transpose(1, 0, 2).astype(np.float16)
            )

        cstk = np.zeros((128, K), dtype=np.float32)
        cstk[:, 0:4] = b1.reshape(4, 128).T
        for s, g in enumerate(cats):
            cstk[:, 4 + s] = b2c_all[g].astype(np.float32)
        cstk[:, 6] = EPS32
        cstk[:, 7 : 7 + F] = inv

        in_maps.append(
            {"xT": xTk, "zT": zTk, "wz1": wz1h, "w2c": w2ck, "cst": cstk}
        )
        slots.append((cats, [int(cnt[g]) for g in cats]))
    return P0, P1, in_maps, slots, idx


def gather_output(P0, P1, slots, idx, core_outs):
    NCH0, NCH1 = P0 // 128, P1 // 128
    out_full = np.zeros(N, dtype=np.float32)
    for k in range(N_CORES):
        om = core_outs[k]  # [128, F]; out[p, colbase+ic] = row soff + ic*128 + p
        cats, counts = slots[k]
        colbase = 0
        for s, g in enumerate(cats):
            nch = (NCH0, NCH1)[s]
            rows_cat = om[:, colbase : colbase + nch].T.reshape(nch * 128)
            n = counts[s]
            if n:
                out_full[idx[g]] = rows_cat[:n]
            colbase += nch
    return out_full


def kernel(x, c, z, W1, b1, W2, b2, Wz, bz, w_s):
    P0, P1, in_maps, slots, idx = prepare(x, c, z, W1, b1, W2, b2, Wz, bz, w_s)
    nc = build_program(P0, P1)
    res = bass_utils.run_bass_kernel_spmd(nc, in_maps, core_ids=list(range(N_CORES)))
    return gather_output(P0, P1, slots, idx, [r["out"] for r in res.results])
